# revision 25
# baseline (speedup 1.0000x reference)
"""Trainium2 Bass kernel for nn_BlockMoEAdapters (8 NeuronCores, SPMD).

Sharding: tokens (B*N = 4096) split contiguously across 8 cores (512 each).
Cores 0-3 hold batch 0, cores 4-7 batch 1. Attention K/V are all-gathered
in fp8 (K in four 2-tile chunks, V in two halves, issued as soon as each
chunk's GEMM completes so the collectives hide under the q GEMM and early
attention pairs); MoE capacity ranks use a tiny 8-core all-gather of
per-core expert counts.

All fat GEMMs run in fp8 e4m3 with perf_mode=DoubleRow (two 128-deep
contraction subtiles per matmul, ~1.5x PE throughput): qkv, v, proj, expert
hidden, dense MLP, and the fused output GEMM. Attention scores run plain
fp8 (contraction 64); the exp writes fp8 probs directly, which feed
DoubleRow attn@V matmuls (V padded 64->72 per head so the subtile stride is
16B-aligned). The router path stays bf16 (top-2 selection is
precision-sensitive); routing-count arithmetic stays exact f32; LayerNorm
stats and broadcasts use f32r (1 cycle/row) matmuls.
"""
import sys

for _p in ('/opt/trn_rl_repo',):
    if _p not in sys.path:
        sys.path.append(_p)

import ml_dtypes
import numpy as np

import concourse.bass as bass
import concourse.mybir as mybir
import concourse.tile as tile
from concourse import bacc
from concourse.bass_utils import run_bass_kernel_spmd

F32 = mybir.dt.float32
F32R = mybir.dt.float32r
BF16 = mybir.dt.bfloat16
F8 = mybir.dt.float8e4
AF = mybir.ActivationFunctionType
ALU = mybir.AluOpType
DR = mybir.MatmulPerfMode.DoubleRow

B, N, D = 2, 2048, 1024
H, HD = 16, 64
E, TOPK = 4, 2
MOEH, MLPH = 256, 4096
T = B * N
NC = 8
TL = T // NC          # 512 tokens per core
NT = TL // 128        # 4 token tiles
DT = D // 128         # 8 channel tiles
CAP = int(T * TOPK / E * 1.0)   # 2048
GRP = 4               # cores per kv-gather group
EPS = 1e-5
VP = 72               # per-head padded v width (64 dims + denom col @64)

_cache = {}


def _mm(nc, out, lhsT, rhs, start, stop, dt=None):
    if dt is not None:
        lhsT, rhs = lhsT.bitcast(dt), rhs.bitcast(dt)
    nc.tensor.matmul(out, lhsT, rhs, start=start, stop=stop)


def _dr(nc, out, lhsT3, rhs3, start, stop):
    nc.tensor.matmul(out, lhsT3, rhs3, start=start, stop=stop, perf_mode=DR)


def _wpair(slab, j):
    # [128, 2, 128] weight pair j from a [128, n*128] lhsT slab
    return slab[:, 2 * j * 128:(2 * j + 2) * 128].rearrange(
        "p (two m) -> p two m", two=2)


def _apair(big, j):
    # [128, 2, TL] activation pair j from a [128, DT*TL] channel-major tile
    return big[:, 2 * j * TL:(2 * j + 2) * TL].rearrange(
        "p (two t) -> p two t", two=2)


def _build():
    nc = bacc.Bacc("TRN2", target_bir_lowering=False, debug=False,
                   num_devices=NC)

    def din(name, shape, dt=F32):
        return nc.dram_tensor(name, list(shape), dt, kind="ExternalInput")

    xT_d = din("xT", (D, TL), BF16)
    noiseT_d = din("noiseT", (E, TL), BF16)
    # host-retiled weight slabs (see _prep_inputs for layouts)
    wqk_d = din("wqk_l", (128, 16 * DT * 128), F8)
    wv_d = din("wv_l", (128, DT * 1024), F8)
    wproj_d = din("wproj_l", (128, DT * DT * 128), F8)
    wmlp1_d = din("wmlp1_l", (128, 32 * DT * 128), BF16)
    we1_d = din("we1_l", (128, 8 * DT * 128), F8)
    wout_d = din("wout_l", (128, DT * 8 * 128), F8)
    wm2_d = din("wm2_l", (128, DT * 32 * 128), BF16)
    wroute_d = din("wroute", (D, E), BF16)
    wnoise_d = din("wnoise", (D, E), BF16)
    be2_d = din("be2", (E, D), BF16)
    ln1g_d = din("ln1g", (D, 1))
    ln1b_d = din("ln1b", (D, 1))
    ln2g_d = din("ln2g", (D, 1))
    ln2b_d = din("ln2b", (D, 1))
    bproj_d = din("bproj", (D, 1))
    broute_d = din("broute", (E, 1))
    bnoise_d = din("bnoise", (E, 1))
    be1_d = din("be1", (E * MOEH, 1))
    bmlp1_d = din("bmlp1", (MLPH, 1))
    bmlp2_d = din("bmlp2", (D, 1))
    ones_d = din("ones128", (128, 128))
    onesb_d = din("ones128b", (128, 128), BF16)
    eye_d = din("eye128", (128, 128))
    utri_d = din("utri128", (128, 128))      # U[s,t] = 1 if s < t
    gsel_d = din("gsel", (E, E * 128), BF16)  # gsel[k, e*128+p] = (k == e)
    wpfx_d = din("wpfx", (NC, 1))            # per-core: 1 for j < core_id

    out_d = nc.dram_tensor("out", [D, TL], F32, kind="ExternalOutput")

    rg_kv = [[0, 1, 2, 3], [4, 5, 6, 7]]
    rg_all = [list(range(NC))]

    with tile.TileContext(nc) as tc:
        with (
            tc.tile_pool(name="dram", bufs=1, space="DRAM") as dpool,
            tc.tile_pool(name="consts", bufs=1) as cpool,
            tc.tile_pool(name="persist", bufs=1) as ppool,
            tc.tile_pool(name="ps_row", bufs=2, space="PSUM") as ps_row,
            tc.tile_pool(name="ps_big", bufs=2, space="PSUM") as ps_big,
            tc.tile_pool(name="ps_bc", bufs=2, space="PSUM") as ps_bc,
            tc.tile_pool(name="wslab", bufs=2) as wpool,
            tc.tile_pool(name="scratch", bufs=2) as spool,
        ):
            # ---------- collective bounce buffers ----------
            k_in = [dpool.tile([256, TL], F8, name=f"k_in{c}")
                    for c in range(4)]
            k_out = [dpool.tile([GRP, 256, TL], F8, name=f"k_out{c}")
                     for c in range(4)]
            v_in = [dpool.tile([128, 4 * 8 * VP], F8, name=f"v_in{h_}")
                    for h_ in range(2)]
            v_out = [dpool.tile([GRP * 128, 4 * 8 * VP], F8,
                                name=f"v_out{h_}") for h_ in range(2)]
            cnt_in = dpool.tile([1, E], F32, name="cnt_in")
            cnt_out = dpool.tile([NC, E], F32, name="cnt_out")

            # ---------- constants (gpsimd queue) ----------
            def load_const(dram, shape, dt=F32):
                t = cpool.tile(list(shape), dt, tag=dram.name, name=dram.name)
                nc.gpsimd.dma_start(t[:], dram[:])
                return t

            ones_sb = load_const(ones_d, (128, 128))
            onesb_sb = load_const(onesb_d, (128, 128), BF16)
            gsel_sb = load_const(gsel_d, (E, E * 128), BF16)
            eye_sb = load_const(eye_d, (128, 128))
            utri_sb = load_const(utri_d, (128, 128))
            wpfx_sb = load_const(wpfx_d, (NC, 1))
            broute_sb = load_const(broute_d, (E, 1))
            bnoise_sb = load_const(bnoise_d, (E, 1))
            be2_sb = load_const(be2_d, (E, D), BF16)
            wv_sb = load_const(wv_d, (128, DT * 1024), F8)

            def load_cols(dram, n, tag):
                # [n*128, 1] dram -> sbuf [128, n] (col j = chunk j)
                t = cpool.tile([128, n], F32, tag=tag, name=tag)
                v = dram[:].rearrange("(a p) b -> a p b", p=128)
                for j in range(n):
                    nc.gpsimd.dma_start(t[:, j:j + 1], v[j])
                return t

            ln1g_sb = load_cols(ln1g_d, DT, "ln1g")
            ln1b_sb = load_cols(ln1b_d, DT, "ln1b")
            ln2g_sb = load_cols(ln2g_d, DT, "ln2g")
            ln2b_sb = load_cols(ln2b_d, DT, "ln2b")
            bproj_sb = load_cols(bproj_d, DT, "bproj")
            be1_sb = load_cols(be1_d, E * MOEH // 128, "be1")
            bmlp1_sb = load_cols(bmlp1_d, MLPH // 128, "bmlp1")
            bmlp2_sb = load_cols(bmlp2_d, DT, "bmlp2")

            # ---------- load x (CM, bf16: LN1 source + residual) ----------
            xT_sb = []
            for j in range(DT):
                t = ppool.tile([128, TL], BF16, tag=f"xT{j}", name=f"xT{j}")
                nc.sync.dma_start(t[:], xT_d[j * 128:(j + 1) * 128, :])
                xT_sb.append(t)

            # big channel-major activation tiles (pairable for DoubleRow)
            x1f8 = ppool.tile([128, DT * TL], F8, tag="x1f8", name="x1f8")
            q8 = ppool.tile([128, DT * TL], F8, tag="q8", name="q8")
            aoT8 = ppool.tile([128, DT * TL], F8, tag="aoT8", name="aoT8")
            x2f8 = ppool.tile([128, DT * TL], F8, tag="x2f8", name="x2f8")
            x2b = ppool.tile([128, DT * TL], BF16, tag="x2b", name="x2b")
            Hmb = ppool.tile([128, (MLPH // 128) * TL], BF16, tag="Hmb",
                             name="Hmb")
            Hmoe8 = ppool.tile([128, 8 * TL], F8, tag="Hmoe8", name="Hmoe8")
            Hg8 = ppool.tile([128, 8 * TL], F8, tag="Hg8", name="Hg8")

            # ---------- LayerNorm in CM -> fp8 (+ optional bf16) ----------
            def layernorm_cm(xtiles, g_sb, b_sb, out8, outb=None, xdt=F32):
                musum = ps_row.tile([1, TL], F32, tag="row", name="musum")
                sqsum = ps_row.tile([1, TL], F32, tag="row", name="sqsum")
                for j in range(DT):
                    if xdt == BF16:
                        xb = xtiles[j]
                    else:
                        xb = spool.tile([128, TL], BF16, tag="lnxb",
                                        name="lnxb", bufs=2)
                        nc.vector.tensor_copy(xb[:], xtiles[j][:])
                    _mm(nc, musum[:], onesb_sb[:, 0:1], xb[:],
                        j == 0, j == DT - 1)
                    sq = spool.tile([128, TL], BF16, tag="lnsq", name="lnsq",
                                    bufs=2)
                    nc.vector.tensor_tensor(sq[:], xb[:], xb[:], ALU.mult)
                    _mm(nc, sqsum[:], onesb_sb[:, 0:1], sq[:],
                        j == 0, j == DT - 1)
                mu = spool.tile([1, TL], F32, tag="lnmu", name="lnmu", bufs=1)
                nc.vector.tensor_scalar_mul(mu[:], musum[:], 1.0 / D)
                msq = spool.tile([1, TL], F32, tag="lnscr", name="lnmsq",
                                 bufs=2)
                nc.vector.tensor_tensor(msq[:], mu[:], mu[:], ALU.mult)
                var = spool.tile([1, TL], F32, tag="lnscr", name="lnvar",
                                 bufs=2)
                nc.vector.scalar_tensor_tensor(var[:], sqsum[:], 1.0 / D,
                                               msq[:], ALU.mult, ALU.subtract)
                vare = spool.tile([1, TL], F32, tag="lnscr", name="lnvare",
                                  bufs=2)
                nc.vector.tensor_scalar_add(vare[:], var[:], EPS)
                sd = spool.tile([1, TL], F32, tag="lnscr", name="lnsd",
                                bufs=2)
                nc.scalar.activation(sd[:], vare[:], AF.Sqrt)
                rsig = spool.tile([1, TL], F32, tag="lnrsig", name="lnrsig",
                                  bufs=1)
                nc.vector.reciprocal_approx_fast(rsig[:], sd[:])
                mu16 = spool.tile([1, TL], BF16, tag="lnmu16", name="lnmu16",
                                  bufs=1)
                nc.vector.tensor_copy(mu16[:], mu[:])
                rs16 = spool.tile([1, TL], BF16, tag="lnrs16", name="lnrs16",
                                  bufs=1)
                nc.vector.tensor_copy(rs16[:], rsig[:])
                mub_ps = ps_bc.tile([128, TL], F32, tag="bc", name="mub")
                _mm(nc, mub_ps[:], onesb_sb[0:1, :], mu16[:], True, True)
                rsb_ps = ps_bc.tile([128, TL], F32, tag="bc", name="rsb")
                _mm(nc, rsb_ps[:], onesb_sb[0:1, :], rs16[:], True, True)
                mub = spool.tile([128, TL], F32, tag="mubsb", name="mubsb",
                                 bufs=1)
                nc.vector.tensor_copy(mub[:], mub_ps[:])
                rsb = spool.tile([128, TL], F32, tag="rsbsb", name="rsbsb",
                                 bufs=1)
                nc.vector.tensor_copy(rsb[:], rsb_ps[:])
                for j in range(DT):
                    t1 = spool.tile([128, TL], F32, tag="lnt1", name="lnt1",
                                    bufs=2)
                    nc.vector.tensor_tensor(t1[:], xtiles[j][:], mub[:],
                                            ALU.subtract)
                    t2 = spool.tile([128, TL], F32, tag="lnt2", name="lnt2",
                                    bufs=2)
                    nc.vector.tensor_tensor(t2[:], t1[:], rsb[:], ALU.mult)
                    nc.vector.tensor_scalar(out8[:, j * TL:(j + 1) * TL],
                                            t2[:], g_sb[:, j:j + 1],
                                            b_sb[:, j:j + 1], ALU.mult,
                                            ALU.add)
                    if outb is not None:
                        nc.vector.tensor_scalar(outb[:, j * TL:(j + 1) * TL],
                                                t2[:], g_sb[:, j:j + 1],
                                                b_sb[:, j:j + 1], ALU.mult,
                                                ALU.add)

            with tc.tile_pool(name="st1", bufs=2) as s1pool:
                layernorm_cm(xT_sb, ln1g_sb, ln1b_sb, x1f8, None, BF16)

                # ---------- k/v GEMMs; chunk collectives pipelined --------
                # collective order k0, v0, k1, k2, k3, v1 matches the order
                # attention pairs consume them (CC processes sequentially)
                def k_tiles(ts):
                    for t in ts:             # k tile t = slab m=8+t
                        slab = wpool.tile([128, DT * 128], F8, tag="qkslab",
                                          name="kslab")
                        nc.sync.dma_start(
                            slab[:], wqk_d[:, (8 + t) * 1024:(9 + t) * 1024])
                        ps = ps_bc.tile([128, TL], F32, tag="bc", name="kps")
                        for j in range(DT // 2):
                            _dr(nc, ps[:], _wpair(slab, j), _apair(x1f8, j),
                                j == 0, j == DT // 2 - 1)
                        ksb = s1pool.tile([128, TL], F8, tag="kevac",
                                          name="kevac", bufs=2)
                        nc.vector.tensor_copy(ksb[:], ps[:])
                        nc.sync.dma_start(
                            k_in[t // 2][(t % 2) * 128:(t % 2 + 1) * 128, :],
                            ksb[:])
                        if t % 2 == 1:
                            nc.gpsimd.collective_compute(
                                "AllGather", ALU.bypass, replica_groups=rg_kv,
                                ins=[k_in[t // 2][:].opt()],
                                outs=[k_out[t // 2][:].opt()])

                def v_half(nn):
                    for mt in range(NT):         # 4 token Mtiles
                        ps = ps_bc.tile([128, 512], F32, tag="bc",
                                        name="vps")
                        for j in range(DT // 2):
                            lhs = x1f8[:, 2 * j * TL:(2 * j + 2) * TL] \
                                .rearrange("p (two t) -> p two t", two=2) \
                                [:, :, mt * 128:(mt + 1) * 128]
                            rhs = wv_sb[:, 2 * j * 1024:(2 * j + 2) * 1024] \
                                .rearrange("p (two c) -> p two c", two=2) \
                                [:, :, nn * 512:(nn + 1) * 512]
                            _dr(nc, ps[:], lhs, rhs, j == 0, j == DT // 2 - 1)
                        vp = s1pool.tile([128, 8 * VP], F8, tag="vpad",
                                         name="vpad", bufs=2)
                        nc.vector.memset(vp[:], 1.0)
                        dst = vp[:].rearrange("p (h c) -> p h c", c=VP)
                        nc.vector.tensor_copy(
                            dst[:, :, 0:64],
                            ps[:].rearrange("p (h c) -> p h c", c=64))
                        nc.sync.dma_start(
                            v_in[nn][:].rearrange(
                                "p (q c) -> p q c",
                                c=8 * VP)[:, :, mt * 2 * VP:(mt + 1) * 2 * VP],
                            vp[:].rearrange("p (q c) -> p q c", c=2 * VP))
                    nc.gpsimd.collective_compute(
                        "AllGather", ALU.bypass, replica_groups=rg_kv,
                        ins=[v_in[nn][:].opt()], outs=[v_out[nn][:].opt()])

                k_tiles([0, 1])
                v_half(0)
                k_tiles([2, 3, 4, 5, 6, 7])
                v_half(1)

                # ---------- q GEMM ----------
                for t in range(DT):
                    slab = wpool.tile([128, DT * 128], F8, tag="qkslab",
                                      name="qslab")
                    nc.sync.dma_start(
                        slab[:], wqk_d[:, t * 1024:(t + 1) * 1024])
                    ps = ps_bc.tile([128, TL], F32, tag="bc", name="qps")
                    for j in range(DT // 2):
                        _dr(nc, ps[:], _wpair(slab, j), _apair(x1f8, j),
                            j == 0, j == DT // 2 - 1)
                    nc.vector.tensor_copy(q8[:, t * TL:(t + 1) * TL], ps[:])

            # ---------- attention (2-head interleave, FD-1024 exp) ----------
            with (
                tc.tile_pool(name="attn", bufs=2) as apool,
                tc.tile_pool(name="vsb", bufs=2) as vpool,
                tc.tile_pool(name="ssb", bufs=4) as spool_s,
            ):
                for p in range(DT):              # head pair
                    hf, pq = p // 4, p % 4       # v half, pair in half
                    kp = []
                    vt = []
                    for r in range(GRP):
                        kt_ = apool.tile([128, TL], F8, tag=f"kp{r}",
                                         name=f"kp{r}")
                        nc.sync.dma_start(
                            kt_[:],
                            k_out[p // 2][r, (p % 2) * 128:(p % 2 + 1) * 128,
                                          :])
                        kp.append(kt_)
                        vt_ = vpool.tile([128, 8 * VP], F8, tag=f"vt{r}",
                                         name=f"vt{r}")
                        nc.gpsimd.dma_start(
                            vt_[:],
                            v_out[hf][r * 128:(r + 1) * 128,
                                      pq * 8 * VP:(pq + 1) * 8 * VP])
                        vt.append(vt_)
                    ao_ps = [ps_bc.tile([VP, TL], F32, tag="bc",
                                        name=f"ao{hh}") for hh in range(2)]
                    for beat in range(8):        # 2 key tiles per beat
                        kt0 = 2 * beat
                        r, cc0 = kt0 // 4, kt0 % 4
                        for hh in range(2):
                            po = 64 * hh
                            s2 = ps_big.tile([128, 2 * TL], F32, tag="big",
                                             name="s2")
                            for u in range(2):
                                cc = cc0 + u
                                _mm(nc, s2[:, u * TL:(u + 1) * TL],
                                    kp[r][po:po + 64,
                                          cc * 128:(cc + 1) * 128],
                                    q8[po:po + 64, p * TL:(p + 1) * TL],
                                    True, True)
                            s_sb = spool_s.tile([128, 2 * TL], F8,
                                                tag="ssb", name="ssb")
                            nc.scalar.activation(s_sb[:], s2[:], AF.Exp,
                                                 scale=0.125)
                            lhs = vt[r][:].rearrange(
                                "p (c h x) -> p c h x", h=2, x=VP) \
                                [:, cc0:cc0 + 2, hh, :]
                            rhs = s_sb[:].rearrange("p (u t) -> p u t", u=2)
                            _dr(nc, ao_ps[hh][:], lhs, rhs,
                                beat == 0, beat == 7)
                    for hh in range(2):
                        po = 64 * hh
                        den = spool_s.tile([1, TL], F32, tag="den",
                                           name="den", bufs=1)
                        nc.vector.tensor_copy(den[:], ao_ps[hh][64:65, :])
                        recip = spool_s.tile([1, TL], F32, tag="recip",
                                             name="recip", bufs=1)
                        nc.vector.reciprocal_approx_fast(recip[:], den[:])
                        recip16 = spool_s.tile([1, TL], BF16, tag="recip16",
                                               name="recip16", bufs=1)
                        nc.vector.tensor_copy(recip16[:], recip[:])
                        bc_ps = ps_row.tile([64, TL], F32, tag="row",
                                            name="aobc")
                        _mm(nc, bc_ps[:], onesb_sb[0:1, 0:64], recip16[:],
                            True, True)
                        bc_sb = spool_s.tile([64, TL], F32, tag="aobcsb",
                                             name="aobcsb", bufs=1)
                        nc.vector.tensor_copy(bc_sb[:], bc_ps[:])
                        nc.vector.tensor_tensor(
                            aoT8[po:po + 64, p * TL:(p + 1) * TL],
                            ao_ps[hh][0:64, :], bc_sb[:], ALU.mult)

                # ---------- proj + residual ----------
                xres = []
                for m in range(DT):
                    slab = wpool.tile([128, DT * 128], F8, tag="qkslab",
                                      name="projslab")
                    nc.sync.dma_start(
                        slab[:], wproj_d[:, m * 1024:(m + 1) * 1024])
                    ps = ps_bc.tile([128, TL], F32, tag="bc", name="proj")
                    for j in range(DT // 2):
                        _dr(nc, ps[:], _wpair(slab, j), _apair(aoT8, j),
                            j == 0, j == DT // 2 - 1)
                    xr = ppool.tile([128, TL], F32, tag=f"xres{m}",
                                    name=f"xres{m}")
                    nc.vector.scalar_tensor_tensor(
                        xr[:], ps[:], bproj_sb[:, m:m + 1], xT_sb[m][:],
                        ALU.add, ALU.add)
                    xres.append(xr)

            # ---------- LN2 (fp8 + bf16 outputs) ----------
            layernorm_cm(xres, ln2g_sb, ln2b_sb, x2f8, x2b)

            # ---------- router (bf16 path) ----------
            logit_ps = ps_row.tile([E, TL], F32, tag="row", name="logit")
            for j in range(DT):
                w = spool.tile([128, E], BF16, tag="wroute", name="wroute")
                nc.sync.dma_start(w[:], wroute_d[j * 128:(j + 1) * 128, :])
                _mm(nc, logit_ps[:], w[:], x2b[:, j * TL:(j + 1) * TL],
                    j == 0, j == DT - 1)
            nlin_ps = ps_row.tile([E, TL], F32, tag="row", name="nlin")
            for j in range(DT):
                w = spool.tile([128, E], BF16, tag="wnoise", name="wnoise")
                nc.sync.dma_start(w[:], wnoise_d[j * 128:(j + 1) * 128, :])
                _mm(nc, nlin_ps[:], w[:], x2b[:, j * TL:(j + 1) * TL],
                    j == 0, j == DT - 1)
            logits = spool.tile([E, TL], F32, tag="logits", name="logits",
                                bufs=1)
            nc.vector.tensor_scalar(logits[:], logit_ps[:],
                                    broute_sb[:, 0:1], None, ALU.add)
            spe = spool.tile([E, TL], BF16, tag="softpe", name="softpe",
                             bufs=1)
            nc.scalar.activation(spe[:], nlin_ps[:], AF.Exp,
                                 bias=bnoise_sb[:, 0:1])
            spe1 = spool.tile([E, TL], BF16, tag="softpe1", name="softpe1",
                              bufs=1)
            nc.vector.tensor_scalar_add(spe1[:], spe[:], 1.0)
            sp = spool.tile([E, TL], BF16, tag="softp", name="softp",
                            bufs=1)
            nc.scalar.activation(sp[:], spe1[:], AF.Ln)
            noiseT_sb = spool.tile([E, TL], BF16, tag="noiseTs",
                                   name="noiseTs", bufs=1)
            nc.sync.dma_start(noiseT_sb[:], noiseT_d[:])
            nsp = spool.tile([E, TL], BF16, tag="nsp", name="nsp", bufs=1)
            nc.vector.tensor_tensor(nsp[:], noiseT_sb[:], sp[:], ALU.mult)
            noisy_cm = spool.tile([E, TL], F32, tag="noisycm", name="noisycm",
                                  bufs=1)
            nc.vector.tensor_tensor(noisy_cm[:], nsp[:], logits[:], ALU.add)

            # ---------- top-2 gates (TM) ----------
            noisy8 = ppool.tile([128, 8 * NT], F32, tag="noisy8",
                                name="noisy8")
            nc.vector.memset(noisy8[:], -1e30)
            m8 = ppool.tile([128, 8 * NT], F32, tag="m8", name="m8")
            gate = ppool.tile([128, E * NT], F32, tag="gate", name="gate")
            mask = ppool.tile([128, E * NT], F32, tag="mask", name="mask")
            geT = ppool.tile([E, TL], F32, tag="geT", name="geT")
            geb = ppool.tile([E, TL], BF16, tag="geb", name="geb")
            cnt_sb = ppool.tile([1, NT * E], F32, tag="cntsb", name="cntsb")
            for j in range(NT):
                tr_ps = ps_bc.tile([128, E], F32, tag="bc", name="ntr")
                nc.tensor.matmul(tr_ps[:],
                                 noisy_cm[:, j * 128:(j + 1) * 128],
                                 eye_sb[0:E, 0:E], is_transpose=True,
                                 start=True, stop=True)
                nc.vector.tensor_copy(noisy8[:, 8 * j:8 * j + E], tr_ps[:])
            for j in range(NT):
                nm = noisy8[:, 8 * j:8 * j + E]
                nc.vector.max(m8[:, 8 * j:8 * j + 8],
                              noisy8[:, 8 * j:8 * j + 8])
                v1 = m8[:, 8 * j:8 * j + 1]
                v2 = m8[:, 8 * j + 1:8 * j + 2]
                oh1 = spool.tile([128, E], F32, tag="oh1", name="oh1")
                nc.vector.tensor_scalar(oh1[:], nm, v1, None, ALU.is_ge)
                msk = mask[:, E * j:E * (j + 1)]
                nc.vector.tensor_scalar(msk, nm, v2, None, ALU.is_ge)
                oh2 = spool.tile([128, E], F32, tag="oh2", name="oh2")
                nc.vector.tensor_tensor(oh2[:], msk, oh1[:], ALU.subtract)
                negv1 = spool.tile([128, 1], F32, tag="negv1", name="negv1")
                nc.vector.tensor_scalar_mul(negv1[:], v1, -1.0)
                p2 = spool.tile([128, 1], F32, tag="p2", name="p2")
                nc.scalar.activation(p2[:], v2, AF.Exp, bias=negv1[:])
                dden = spool.tile([128, 1], F32, tag="dden", name="dden")
                nc.vector.tensor_scalar_add(dden[:], p2[:], 1.0)
                rd = spool.tile([128, 1], F32, tag="rd", name="rd")
                nc.vector.reciprocal(rd[:], dden[:])
                gnum = spool.tile([128, E], F32, tag="gnum", name="gnum")
                nc.vector.tensor_scalar(gnum[:], oh2[:], p2[:], None,
                                        ALU.mult)
                gnum2 = spool.tile([128, E], F32, tag="gnum2", name="gnum2")
                nc.vector.tensor_tensor(gnum2[:], gnum[:], oh1[:], ALU.add)
                nc.vector.tensor_scalar(gate[:, E * j:E * (j + 1)],
                                        gnum2[:], rd[:], None, ALU.mult)
                cps = ps_row.tile([1, E], F32, tag="row", name="cnt")
                _mm(nc, cps[:], ones_sb[:, 0:1], msk, True, True, F32)
                nc.vector.tensor_copy(cnt_sb[0:1, E * j:E * (j + 1)], cps[:])

            # total counts -> all-gather
            tot = spool.tile([1, E], F32, tag="cnttot", name="cnttot",
                             bufs=1)
            nc.vector.tensor_tensor(tot[:], cnt_sb[0:1, 0:E],
                                    cnt_sb[0:1, E:2 * E], ALU.add)
            nc.vector.tensor_tensor(tot[:], tot[:], cnt_sb[0:1, 2 * E:3 * E],
                                    ALU.add)
            nc.vector.tensor_tensor(tot[:], tot[:], cnt_sb[0:1, 3 * E:4 * E],
                                    ALU.add)
            nc.sync.dma_start(cnt_in[:], tot[:])
            nc.gpsimd.collective_compute(
                "AllGather", ALU.bypass, replica_groups=rg_all,
                ins=[cnt_in[:].opt()], outs=[cnt_out[:].opt()])

            # ---------- MoE hidden + MLP hidden (overlaps counts AG) ------
            for e in range(E):
                for hmi in range(MOEH // 128):
                    me = 2 * e + hmi
                    slab = wpool.tile([128, DT * 128], F8, tag="qkslab",
                                      name="e1slab")
                    nc.sync.dma_start(
                        slab[:], we1_d[:, me * 1024:(me + 1) * 1024])
                    ps = ps_bc.tile([128, TL], F32, tag="bc", name="hmoe")
                    for j in range(DT // 2):
                        _dr(nc, ps[:], _wpair(slab, j), _apair(x2f8, j),
                            j == 0, j == DT // 2 - 1)
                    nc.scalar.activation(
                        Hmoe8[:, me * TL:(me + 1) * TL], ps[:], AF.Gelu,
                        bias=be1_sb[:, me:me + 1])
            for m in range(MLPH // 128):
                slab = wpool.tile([128, DT * 128], BF16, tag="m1slab",
                                  name="m1slab")
                nc.sync.dma_start(
                    slab[:], wmlp1_d[:, m * 1024:(m + 1) * 1024])
                ps = ps_bc.tile([128, TL], F32, tag="bc", name="hm")
                for kk in range(DT):
                    _mm(nc, ps[:], slab[:, kk * 128:(kk + 1) * 128],
                        x2b[:, kk * TL:(kk + 1) * TL],
                        kk == 0, kk == DT - 1)
                nc.scalar.activation(Hmb[:, m * TL:(m + 1) * TL], ps[:],
                                     AF.Gelu, bias=bmlp1_sb[:, m:m + 1])

            # ---------- ranks / keep / gate_eff ----------
            cntg = spool.tile([NC, E], F32, tag="cntg", name="cntg", bufs=1)
            nc.sync.dma_start(cntg[:], cnt_out[:])
            off_ps = ps_row.tile([1, E], F32, tag="row", name="off")
            _mm(nc, off_ps[:], wpfx_sb[:], cntg[:], True, True, F32)
            car = spool.tile([1, E * NT], F32, tag="car", name="car", bufs=1)
            nc.vector.tensor_copy(car[:, 0:E], off_ps[:])
            for j in range(1, NT):
                nc.vector.tensor_tensor(car[:, E * j:E * (j + 1)],
                                        car[:, E * (j - 1):E * j],
                                        cnt_sb[0:1, E * (j - 1):E * j],
                                        ALU.add)
            ge_tm = ppool.tile([128, E * NT], F32, tag="getm", name="getm")
            for j in range(NT):
                rk_ps = ps_bc.tile([128, E], F32, tag="bc", name="rank")
                _mm(nc, rk_ps[:], utri_sb[:],
                    mask[:, E * j:E * (j + 1)], True, False, F32)
                _mm(nc, rk_ps[:], ones_sb[0:1, :],
                    car[:, E * j:E * (j + 1)], False, True, F32)
                keep = spool.tile([128, E], F32, tag="keep", name="keep")
                nc.vector.tensor_scalar(keep[:], rk_ps[:], float(CAP), None,
                                        ALU.is_lt)
                nc.vector.tensor_tensor(ge_tm[:, E * j:E * (j + 1)],
                                        gate[:, E * j:E * (j + 1)],
                                        keep[:], ALU.mult)
            for j in range(NT):
                tr_ps = ps_bc.tile([E, 128], F32, tag="bc", name="getr")
                nc.tensor.matmul(tr_ps[:], ge_tm[:, E * j:E * (j + 1)],
                                 eye_sb[:, :], is_transpose=True,
                                 start=True, stop=True)
                nc.vector.tensor_copy(geT[:, j * 128:(j + 1) * 128], tr_ps[:])
            nc.vector.tensor_copy(geb[:], geT[:])

            # gate the MoE hidden
            for e in range(E):
                bc_ps = ps_bc.tile([128, TL], F32, tag="bc", name="gbc")
                _mm(nc, bc_ps[:], gsel_sb[:, e * 128:(e + 1) * 128],
                    geb[:], True, True)
                bc_sb = spool.tile([128, TL], BF16, tag="gbcsb", name="gbcsb",
                                   bufs=2)
                nc.vector.tensor_copy(bc_sb[:], bc_ps[:])
                for hmi in range(MOEH // 128):
                    me = 2 * e + hmi
                    nc.vector.tensor_tensor(Hg8[:, me * TL:(me + 1) * TL],
                                            Hmoe8[:, me * TL:(me + 1) * TL],
                                            bc_sb[:], ALU.mult)

            # ---------- output GEMM: moe + be2 + mlp, fused accum ----------
            for m in range(DT):
                slab = wpool.tile([128, 8 * 128], F8, tag="outslab",
                                  name="outslab")
                nc.sync.dma_start(
                    slab[:], wout_d[:, m * 1024:(m + 1) * 1024])
                slab2 = wpool.tile([128, 32 * 128], BF16, tag="out2slab",
                                   name="out2slab")
                nc.sync.dma_start(
                    slab2[:], wm2_d[:, m * 4096:(m + 1) * 4096])
                ps = ps_bc.tile([128, TL], F32, tag="bc", name="out")
                for j in range(4):           # we2 tile pairs (e, hmi)
                    _dr(nc, ps[:], _wpair(slab, j), _apair(Hg8, j),
                        j == 0, False)
                _mm(nc, ps[:], be2_sb[:, m * 128:(m + 1) * 128],
                    geb[:], False, False)
                for kk in range(32):         # mlp2 tiles (bf16)
                    _mm(nc, ps[:], slab2[:, kk * 128:(kk + 1) * 128],
                        Hmb[:, kk * TL:(kk + 1) * TL],
                        False, kk == 31)
                o = spool.tile([128, TL], F32, tag="outsb", name="outsb",
                               bufs=2)
                nc.vector.scalar_tensor_tensor(
                    o[:], ps[:], bmlp2_sb[:, m:m + 1], xres[m][:],
                    ALU.add, ALU.add)
                nc.sync.dma_start(out_d[m * 128:(m + 1) * 128, :], o[:])

    nc.compile()
    return nc


def _tile_lhst(w, n_k, n_m):
    # w: [n_k*128, n_m*128] -> [128, n_m, n_k, 128] -> [128, n_m*n_k*128]
    kdim, mdim = w.shape
    return np.ascontiguousarray(
        w.reshape(n_k, 128, n_m, 128).transpose(1, 2, 0, 3)
        .reshape(128, n_m * n_k * 128))


def _prep_inputs(inputs):
    f32 = lambda a: np.ascontiguousarray(np.asarray(a, np.float32))
    bf = lambda a: np.ascontiguousarray(
        np.asarray(a, np.float32).astype(ml_dtypes.bfloat16))
    f8 = lambda a: np.ascontiguousarray(
        np.clip(np.asarray(a, np.float32), -240, 240)
        .astype(ml_dtypes.float8_e4m3))
    x = f32(inputs["x"]).reshape(T, D)
    noise = f32(inputs["noise"]).reshape(T, E)
    w_qkv = np.asarray(inputs["w_qkv"], np.float32)
    wqkT = w_qkv[:2 * D].T                       # [D, 2048]
    wvT = w_qkv[2 * D:].T                        # [D, D]
    wprojT = np.asarray(inputs["w_proj"], np.float32).T
    we1 = np.asarray(inputs["we1"], np.float32)  # [E, D, MOEH]
    we2 = np.asarray(inputs["we2"], np.float32)  # [E, MOEH, D]
    wmlp1 = np.asarray(inputs["w_mlp1"], np.float32)   # [D, MLPH]
    wmlp2 = np.asarray(inputs["w_mlp2"], np.float32)   # [MLPH, D]

    # we1 slabs: m-index = e*2+hmi over [D, 256] each
    we1_flat = np.concatenate([we1[e] for e in range(E)], 1)  # [D, E*MOEH]
    # wout: per m, 8 we2 tiles (e,hmi); wm2: per m, 32 wmlp2 tiles
    we2_l = we2.reshape(E, 2, 128, DT, 128).transpose(2, 3, 0, 1, 4) \
        .reshape(128, DT * 8 * 128)
    wm2_l = wmlp2.reshape(32, 128, DT, 128).transpose(1, 2, 0, 3) \
        .reshape(128, DT * 32 * 128)

    shared = dict(
        wqk_l=f8(_tile_lhst(wqkT, DT, 16)),
        wv_l=f8(np.ascontiguousarray(
            wvT.reshape(DT, 128, D).transpose(1, 0, 2).reshape(128, DT * D))),
        wproj_l=f8(_tile_lhst(wprojT, DT, DT)),
        wmlp1_l=bf(_tile_lhst(wmlp1, DT, 32)),
        we1_l=f8(_tile_lhst(we1_flat, DT, 8)),
        wout_l=f8(we2_l),
        wm2_l=bf(wm2_l),
        wroute=bf(inputs["w_route"]),
        wnoise=bf(inputs["w_noise"]),
        be2=bf(inputs["be2"]),
        ln1g=f32(inputs["ln1_g"]).reshape(D, 1),
        ln1b=f32(inputs["ln1_b"]).reshape(D, 1),
        ln2g=f32(inputs["ln2_g"]).reshape(D, 1),
        ln2b=f32(inputs["ln2_b"]).reshape(D, 1),
        bproj=f32(inputs["b_proj"]).reshape(D, 1),
        broute=f32(inputs["b_route"]).reshape(E, 1),
        bnoise=f32(inputs["b_noise"]).reshape(E, 1),
        be1=f32(inputs["be1"]).reshape(E * MOEH, 1),
        bmlp1=f32(inputs["b_mlp1"]).reshape(MLPH, 1),
        bmlp2=f32(inputs["b_mlp2"]).reshape(D, 1),
        ones128=np.ones((128, 128), np.float32),
        eye128=np.eye(128, dtype=np.float32),
        utri128=np.triu(np.ones((128, 128), np.float32), 1),
        gsel=np.repeat(np.eye(E, dtype=np.float32), 128, 1)
            .astype(ml_dtypes.bfloat16),
        ones128b=np.ones((128, 128), ml_dtypes.bfloat16),
    )
    in_maps = []
    for c in range(NC):
        m = dict(shared)
        m["xT"] = bf(x[c * TL:(c + 1) * TL].T)
        m["noiseT"] = bf(noise[c * TL:(c + 1) * TL].T)
        m["wpfx"] = (np.arange(NC) < c).astype(np.float32).reshape(NC, 1)
        in_maps.append(m)
    return in_maps


def _run(inputs, trace=False):
    if "nc" not in _cache:
        _cache["nc"] = _build()
    nc = _cache["nc"]
    in_maps = _prep_inputs(inputs)
    res = run_bass_kernel_spmd(nc, in_maps, core_ids=list(range(NC)),
                               trace=trace)
    _cache["last_res"] = res
    shards = [res.results[c]["out"] for c in range(NC)]   # each [D, TL]
    out = np.concatenate([np.asarray(s, np.float32).T for s in shards],
                         0).reshape(B, N, D)
    return out.astype(np.float32), res.exec_time_ns


def kernel(**inputs):
    out, _ = _run(inputs, trace=False)
    return out


# revision 28
# speedup vs baseline: 1.0368x; 1.0368x over previous
"""Trainium2 Bass kernel for nn_BlockMoEAdapters (8 NeuronCores, SPMD).

Sharding: tokens (B*N = 4096) split contiguously across 8 cores (512 each).
Cores 0-3 hold batch 0, cores 4-7 batch 1. Attention K/V are all-gathered
in fp8 (K in four 2-tile chunks, V in two halves, issued as soon as each
chunk's GEMM completes so the collectives hide under the q GEMM and early
attention pairs); MoE capacity ranks use a tiny 8-core all-gather of
per-core expert counts.

All fat GEMMs run in fp8 e4m3 with perf_mode=DoubleRow (two 128-deep
contraction subtiles per matmul, ~1.5x PE throughput): qkv, v, proj, expert
hidden, dense MLP, and the fused output GEMM. Attention scores run plain
fp8 (contraction 64); the exp writes fp8 probs directly, which feed
DoubleRow attn@V matmuls (V padded 64->72 per head so the subtile stride is
16B-aligned). The router path stays bf16 (top-2 selection is
precision-sensitive); routing-count arithmetic stays exact f32; LayerNorm
stats and broadcasts use f32r (1 cycle/row) matmuls.
"""
import sys

for _p in ('/opt/trn_rl_repo',):
    if _p not in sys.path:
        sys.path.append(_p)

import ml_dtypes
import numpy as np

import concourse.bass as bass
import concourse.mybir as mybir
import concourse.tile as tile
from concourse import bacc
from concourse.bass_utils import run_bass_kernel_spmd

F32 = mybir.dt.float32
F32R = mybir.dt.float32r
BF16 = mybir.dt.bfloat16
F8 = mybir.dt.float8e4
AF = mybir.ActivationFunctionType
ALU = mybir.AluOpType
DR = mybir.MatmulPerfMode.DoubleRow

B, N, D = 2, 2048, 1024
H, HD = 16, 64
E, TOPK = 4, 2
MOEH, MLPH = 256, 4096
T = B * N
NC = 8
TL = T // NC          # 512 tokens per core
NT = TL // 128        # 4 token tiles
DT = D // 128         # 8 channel tiles
CAP = int(T * TOPK / E * 1.0)   # 2048
GRP = 4               # cores per kv-gather group
EPS = 1e-5
VP = 72               # per-head padded v width (64 dims + denom col @64)

_cache = {}


def _mm(nc, out, lhsT, rhs, start, stop, dt=None):
    if dt is not None:
        lhsT, rhs = lhsT.bitcast(dt), rhs.bitcast(dt)
    nc.tensor.matmul(out, lhsT, rhs, start=start, stop=stop)


def _dr(nc, out, lhsT3, rhs3, start, stop):
    nc.tensor.matmul(out, lhsT3, rhs3, start=start, stop=stop, perf_mode=DR)


def _wpair(slab, j):
    # [128, 2, 128] weight pair j from a [128, n*128] lhsT slab
    return slab[:, 2 * j * 128:(2 * j + 2) * 128].rearrange(
        "p (two m) -> p two m", two=2)


def _apair(big, j):
    # [128, 2, TL] activation pair j from a [128, DT*TL] channel-major tile
    return big[:, 2 * j * TL:(2 * j + 2) * TL].rearrange(
        "p (two t) -> p two t", two=2)


def _build():
    nc = bacc.Bacc("TRN2", target_bir_lowering=False, debug=False,
                   num_devices=NC)

    def din(name, shape, dt=F32):
        return nc.dram_tensor(name, list(shape), dt, kind="ExternalInput")

    xT_d = din("xT", (D, TL), BF16)
    noiseT_d = din("noiseT", (E, TL), BF16)
    # host-retiled weight slabs (see _prep_inputs for layouts)
    wqk_d = din("wqk_l", (128, 16 * DT * 128), F8)
    wv_d = din("wv_l", (128, DT * 1024), F8)
    wproj_d = din("wproj_l", (128, DT * DT * 128), F8)
    wmlp1_d = din("wmlp1_l", (128, 32 * DT * 128), BF16)
    we1_d = din("we1_l", (128, 8 * DT * 128), F8)
    wout_d = din("wout_l", (128, DT * 8 * 128), F8)
    wm2_d = din("wm2_l", (128, DT * 32 * 128), BF16)
    wroute_d = din("wroute", (D, E), BF16)
    wnoise_d = din("wnoise", (D, E), BF16)
    be2_d = din("be2", (E, D), BF16)
    ln1g_d = din("ln1g", (D, 1))
    ln1b_d = din("ln1b", (D, 1))
    ln2g_d = din("ln2g", (D, 1))
    ln2b_d = din("ln2b", (D, 1))
    bproj_d = din("bproj", (D, 1))
    broute_d = din("broute", (E, 1))
    bnoise_d = din("bnoise", (E, 1))
    be1_d = din("be1", (E * MOEH, 1))
    bmlp1_d = din("bmlp1", (MLPH, 1))
    bmlp2_d = din("bmlp2", (D, 1))
    ones_d = din("ones128", (128, 128))
    onesb_d = din("ones128b", (128, 128), BF16)
    eye_d = din("eye128", (128, 128))
    utri_d = din("utri128", (128, 128))      # U[s,t] = 1 if s < t
    gsel_d = din("gsel", (E, E * 128), BF16)  # gsel[k, e*128+p] = (k == e)
    wpfx_d = din("wpfx", (NC, 1))            # per-core: 1 for j < core_id

    out_d = nc.dram_tensor("out", [D, TL], F32, kind="ExternalOutput")

    rg_kv = [[0, 1, 2, 3], [4, 5, 6, 7]]
    rg_all = [list(range(NC))]

    with tile.TileContext(nc) as tc:
        with (
            tc.tile_pool(name="dram", bufs=1, space="DRAM") as dpool,
            tc.tile_pool(name="consts", bufs=1) as cpool,
            tc.tile_pool(name="persist", bufs=1) as ppool,
            tc.tile_pool(name="ps_row", bufs=2, space="PSUM") as ps_row,
            tc.tile_pool(name="ps_big", bufs=2, space="PSUM") as ps_big,
            tc.tile_pool(name="ps_bc", bufs=2, space="PSUM") as ps_bc,
            tc.tile_pool(name="wslab", bufs=2) as wpool,
            tc.tile_pool(name="scratch", bufs=2) as spool,
        ):
            # ---------- collective bounce buffers ----------
            k_in = [dpool.tile([256, TL], F8, name=f"k_in{c}")
                    for c in range(4)]
            k_out = [dpool.tile([GRP, 256, TL], F8, name=f"k_out{c}")
                     for c in range(4)]
            v_in = [dpool.tile([128, 4 * 8 * VP], F8, name=f"v_in{h_}")
                    for h_ in range(2)]
            v_out = [dpool.tile([GRP * 128, 4 * 8 * VP], F8,
                                name=f"v_out{h_}") for h_ in range(2)]
            cnt_in = dpool.tile([1, E], F32, name="cnt_in")
            cnt_out = dpool.tile([NC, E], F32, name="cnt_out")

            # ---------- constants (scalar queue; gpsimd stays clear for
            # the collectives so the kv all-gathers trigger early) ----------
            def load_const(dram, shape, dt=F32):
                t = cpool.tile(list(shape), dt, tag=dram.name, name=dram.name)
                nc.scalar.dma_start(t[:], dram[:])
                return t

            ones_sb = load_const(ones_d, (128, 128))
            onesb_sb = load_const(onesb_d, (128, 128), BF16)
            gsel_sb = load_const(gsel_d, (E, E * 128), BF16)
            eye_sb = load_const(eye_d, (128, 128))
            utri_sb = load_const(utri_d, (128, 128))
            wpfx_sb = load_const(wpfx_d, (NC, 1))
            broute_sb = load_const(broute_d, (E, 1))
            bnoise_sb = load_const(bnoise_d, (E, 1))
            be2_sb = load_const(be2_d, (E, D), BF16)
            wv_sb = load_const(wv_d, (128, DT * 1024), F8)

            def load_cols(dram, n, tag):
                # [n*128, 1] dram -> sbuf [128, n] (col j = chunk j)
                t = cpool.tile([128, n], F32, tag=tag, name=tag)
                nc.scalar.dma_start(
                    t[:], dram[:].rearrange("(a p) b -> p (a b)", p=128))
                return t

            ln1g_sb = load_cols(ln1g_d, DT, "ln1g")
            ln1b_sb = load_cols(ln1b_d, DT, "ln1b")
            ln2g_sb = load_cols(ln2g_d, DT, "ln2g")
            ln2b_sb = load_cols(ln2b_d, DT, "ln2b")
            bproj_sb = load_cols(bproj_d, DT, "bproj")
            be1_sb = load_cols(be1_d, E * MOEH // 128, "be1")
            bmlp1_sb = load_cols(bmlp1_d, MLPH // 128, "bmlp1")
            bmlp2_sb = load_cols(bmlp2_d, DT, "bmlp2")

            # ---------- load x (CM, bf16: LN1 source + residual) ----------
            xT_sb = []
            for j in range(DT):
                t = ppool.tile([128, TL], BF16, tag=f"xT{j}", name=f"xT{j}")
                nc.sync.dma_start(t[:], xT_d[j * 128:(j + 1) * 128, :])
                xT_sb.append(t)

            # big channel-major activation tiles (pairable for DoubleRow)
            x1f8 = ppool.tile([128, DT * TL], F8, tag="x1f8", name="x1f8")
            q8 = ppool.tile([128, DT * TL], F8, tag="q8", name="q8")
            aoT8 = ppool.tile([128, DT * TL], F8, tag="aoT8", name="aoT8")
            x2f8 = ppool.tile([128, DT * TL], F8, tag="x2f8", name="x2f8")
            x2b = ppool.tile([128, DT * TL], BF16, tag="x2b", name="x2b")
            Hmb = ppool.tile([128, (MLPH // 128) * TL], BF16, tag="Hmb",
                             name="Hmb")
            Hmoe8 = ppool.tile([128, 8 * TL], F8, tag="Hmoe8", name="Hmoe8")
            Hg8 = ppool.tile([128, 8 * TL], F8, tag="Hg8", name="Hg8")

            # ---------- LayerNorm in CM -> fp8 (+ optional bf16) ----------
            def layernorm_cm(xtiles, g_sb, b_sb, out8, outb=None, xdt=F32):
                musum = ps_row.tile([1, TL], F32, tag="row", name="musum")
                sqsum = ps_row.tile([1, TL], F32, tag="row", name="sqsum")
                for j in range(DT):
                    if xdt == BF16:
                        xb = xtiles[j]
                    else:
                        xb = spool.tile([128, TL], BF16, tag="lnxb",
                                        name="lnxb", bufs=2)
                        nc.vector.tensor_copy(xb[:], xtiles[j][:])
                    _mm(nc, musum[:], onesb_sb[:, 0:1], xb[:],
                        j == 0, j == DT - 1)
                    sq = spool.tile([128, TL], BF16, tag="lnsq", name="lnsq",
                                    bufs=2)
                    nc.vector.tensor_tensor(sq[:], xb[:], xb[:], ALU.mult)
                    _mm(nc, sqsum[:], onesb_sb[:, 0:1], sq[:],
                        j == 0, j == DT - 1)
                mu = spool.tile([1, TL], F32, tag="lnmu", name="lnmu", bufs=1)
                nc.vector.tensor_scalar_mul(mu[:], musum[:], 1.0 / D)
                msq = spool.tile([1, TL], F32, tag="lnscr", name="lnmsq",
                                 bufs=2)
                nc.vector.tensor_tensor(msq[:], mu[:], mu[:], ALU.mult)
                var = spool.tile([1, TL], F32, tag="lnscr", name="lnvar",
                                 bufs=2)
                nc.vector.scalar_tensor_tensor(var[:], sqsum[:], 1.0 / D,
                                               msq[:], ALU.mult, ALU.subtract)
                vare = spool.tile([1, TL], F32, tag="lnscr", name="lnvare",
                                  bufs=2)
                nc.vector.tensor_scalar_add(vare[:], var[:], EPS)
                sd = spool.tile([1, TL], F32, tag="lnscr", name="lnsd",
                                bufs=2)
                nc.scalar.activation(sd[:], vare[:], AF.Sqrt)
                rsig = spool.tile([1, TL], F32, tag="lnrsig", name="lnrsig",
                                  bufs=1)
                nc.vector.reciprocal_approx_fast(rsig[:], sd[:])
                mu16 = spool.tile([1, TL], BF16, tag="lnmu16", name="lnmu16",
                                  bufs=1)
                nc.vector.tensor_copy(mu16[:], mu[:])
                rs16 = spool.tile([1, TL], BF16, tag="lnrs16", name="lnrs16",
                                  bufs=1)
                nc.vector.tensor_copy(rs16[:], rsig[:])
                mub_ps = ps_bc.tile([128, TL], F32, tag="bc", name="mub")
                _mm(nc, mub_ps[:], onesb_sb[0:1, :], mu16[:], True, True)
                rsb_ps = ps_bc.tile([128, TL], F32, tag="bc", name="rsb")
                _mm(nc, rsb_ps[:], onesb_sb[0:1, :], rs16[:], True, True)
                mub = spool.tile([128, TL], F32, tag="mubsb", name="mubsb",
                                 bufs=1)
                nc.vector.tensor_copy(mub[:], mub_ps[:])
                rsb = spool.tile([128, TL], F32, tag="rsbsb", name="rsbsb",
                                 bufs=1)
                nc.vector.tensor_copy(rsb[:], rsb_ps[:])
                for j in range(DT):
                    t1 = spool.tile([128, TL], F32, tag="lnt1", name="lnt1",
                                    bufs=2)
                    nc.vector.tensor_tensor(t1[:], xtiles[j][:], mub[:],
                                            ALU.subtract)
                    t2 = spool.tile([128, TL], F32, tag="lnt2", name="lnt2",
                                    bufs=2)
                    nc.vector.tensor_tensor(t2[:], t1[:], rsb[:], ALU.mult)
                    nc.vector.tensor_scalar(out8[:, j * TL:(j + 1) * TL],
                                            t2[:], g_sb[:, j:j + 1],
                                            b_sb[:, j:j + 1], ALU.mult,
                                            ALU.add)
                    if outb is not None:
                        nc.vector.tensor_scalar(outb[:, j * TL:(j + 1) * TL],
                                                t2[:], g_sb[:, j:j + 1],
                                                b_sb[:, j:j + 1], ALU.mult,
                                                ALU.add)

            with tc.tile_pool(name="st1", bufs=2) as s1pool:
                layernorm_cm(xT_sb, ln1g_sb, ln1b_sb, x1f8, None, BF16)

                # ---------- k/v GEMMs; chunk collectives pipelined --------
                # collective order k0, v0, k1, k2, k3, v1 matches the order
                # attention pairs consume them (CC processes sequentially)
                def k_tiles(ts):
                    for t in ts:             # k tile t = slab m=8+t
                        slab = wpool.tile([128, DT * 128], F8, tag="qkslab",
                                          name="kslab")
                        nc.sync.dma_start(
                            slab[:], wqk_d[:, (8 + t) * 1024:(9 + t) * 1024])
                        ps = ps_bc.tile([128, TL], F32, tag="bc", name="kps")
                        for j in range(DT // 2):
                            _dr(nc, ps[:], _wpair(slab, j), _apair(x1f8, j),
                                j == 0, j == DT // 2 - 1)
                        ksb = s1pool.tile([128, TL], F8, tag="kevac",
                                          name="kevac", bufs=2)
                        nc.vector.tensor_copy(ksb[:], ps[:])
                        nc.sync.dma_start(
                            k_in[t // 2][(t % 2) * 128:(t % 2 + 1) * 128, :],
                            ksb[:])
                        if t % 2 == 1:
                            nc.gpsimd.collective_compute(
                                "AllGather", ALU.bypass, replica_groups=rg_kv,
                                ins=[k_in[t // 2][:].opt()],
                                outs=[k_out[t // 2][:].opt()])

                def v_half(nn):
                    for mt in range(NT):         # 4 token Mtiles
                        ps = ps_bc.tile([128, 512], F32, tag="bc",
                                        name="vps")
                        for j in range(DT // 2):
                            lhs = x1f8[:, 2 * j * TL:(2 * j + 2) * TL] \
                                .rearrange("p (two t) -> p two t", two=2) \
                                [:, :, mt * 128:(mt + 1) * 128]
                            rhs = wv_sb[:, 2 * j * 1024:(2 * j + 2) * 1024] \
                                .rearrange("p (two c) -> p two c", two=2) \
                                [:, :, nn * 512:(nn + 1) * 512]
                            _dr(nc, ps[:], lhs, rhs, j == 0, j == DT // 2 - 1)
                        vp = s1pool.tile([128, 8 * VP], F8, tag="vpad",
                                         name="vpad", bufs=2)
                        nc.vector.memset(vp[:], 1.0)
                        dst = vp[:].rearrange("p (h c) -> p h c", c=VP)
                        nc.vector.tensor_copy(
                            dst[:, :, 0:64],
                            ps[:].rearrange("p (h c) -> p h c", c=64))
                        nc.sync.dma_start(
                            v_in[nn][:].rearrange(
                                "p (q c) -> p q c",
                                c=8 * VP)[:, :, mt * 2 * VP:(mt + 1) * 2 * VP],
                            vp[:].rearrange("p (q c) -> p q c", c=2 * VP))
                    nc.gpsimd.collective_compute(
                        "AllGather", ALU.bypass, replica_groups=rg_kv,
                        ins=[v_in[nn][:].opt()], outs=[v_out[nn][:].opt()])

                k_tiles([0, 1])
                v_half(0)
                k_tiles([2, 3, 4, 5, 6, 7])
                v_half(1)

                # ---------- q GEMM ----------
                for t in range(DT):
                    slab = wpool.tile([128, DT * 128], F8, tag="qkslab",
                                      name="qslab")
                    nc.sync.dma_start(
                        slab[:], wqk_d[:, t * 1024:(t + 1) * 1024])
                    ps = ps_bc.tile([128, TL], F32, tag="bc", name="qps")
                    for j in range(DT // 2):
                        _dr(nc, ps[:], _wpair(slab, j), _apair(x1f8, j),
                            j == 0, j == DT // 2 - 1)
                    nc.vector.tensor_copy(q8[:, t * TL:(t + 1) * TL], ps[:])

            # ---------- attention (2-head interleave, FD-1024 exp) ----------
            with (
                tc.tile_pool(name="attn", bufs=2) as apool,
                tc.tile_pool(name="vsb", bufs=2) as vpool,
                tc.tile_pool(name="ssb", bufs=4) as spool_s,
            ):
                for p in range(DT):              # head pair
                    hf, pq = p // 4, p % 4       # v half, pair in half
                    kp = []
                    vt = []
                    for r in range(GRP):
                        kt_ = apool.tile([128, TL], F8, tag=f"kp{r}",
                                         name=f"kp{r}")
                        nc.sync.dma_start(
                            kt_[:],
                            k_out[p // 2][r, (p % 2) * 128:(p % 2 + 1) * 128,
                                          :])
                        kp.append(kt_)
                        vt_ = vpool.tile([128, 8 * VP], F8, tag=f"vt{r}",
                                         name=f"vt{r}")
                        nc.gpsimd.dma_start(
                            vt_[:],
                            v_out[hf][r * 128:(r + 1) * 128,
                                      pq * 8 * VP:(pq + 1) * 8 * VP])
                        vt.append(vt_)
                    ao_ps = [ps_bc.tile([VP, TL], F32, tag="bc",
                                        name=f"ao{hh}") for hh in range(2)]
                    for beat in range(8):        # 2 key tiles per beat
                        kt0 = 2 * beat
                        r, cc0 = kt0 // 4, kt0 % 4
                        # scores: alternate hh so consecutive matmuls hit
                        # disjoint PE row groups (rows 0-63 vs 64-127) and
                        # run concurrently in the 32x32 subarrays
                        s2 = {}
                        for u in range(2):
                            cc = cc0 + u
                            for hh in range(2):
                                po = 64 * hh
                                ps = ps_big.tile([128, TL], F32, tag="s2h",
                                                 name="s2h", bufs=4)
                                _mm(nc, ps[:],
                                    kp[r][po:po + 64,
                                          cc * 128:(cc + 1) * 128],
                                    q8[po:po + 64, p * TL:(p + 1) * TL],
                                    True, True)
                                s2[(hh, u)] = ps
                        s_sb = [spool_s.tile([128, 2 * TL], F8,
                                             tag="ssb", name="ssb")
                                for _ in range(2)]
                        for u in range(2):
                            for hh in range(2):
                                nc.scalar.activation(
                                    s_sb[hh][:, u * TL:(u + 1) * TL],
                                    s2[(hh, u)][:], AF.Exp, scale=0.125)
                        for hh in range(2):
                            lhs = vt[r][:].rearrange(
                                "p (c h x) -> p c h x", h=2, x=VP) \
                                [:, cc0:cc0 + 2, hh, :]
                            rhs = s_sb[hh][:].rearrange(
                                "p (u t) -> p u t", u=2)
                            _dr(nc, ao_ps[hh][:], lhs, rhs,
                                beat == 0, beat == 7)
                    for hh in range(2):
                        po = 64 * hh
                        den = spool_s.tile([1, TL], F32, tag="den",
                                           name="den", bufs=1)
                        nc.vector.tensor_copy(den[:], ao_ps[hh][64:65, :])
                        recip = spool_s.tile([1, TL], F32, tag="recip",
                                             name="recip", bufs=1)
                        nc.vector.reciprocal_approx_fast(recip[:], den[:])
                        recip16 = spool_s.tile([1, TL], BF16, tag="recip16",
                                               name="recip16", bufs=1)
                        nc.vector.tensor_copy(recip16[:], recip[:])
                        bc_ps = ps_row.tile([64, TL], F32, tag="row",
                                            name="aobc")
                        _mm(nc, bc_ps[:], onesb_sb[0:1, 0:64], recip16[:],
                            True, True)
                        bc_sb = spool_s.tile([64, TL], F32, tag="aobcsb",
                                             name="aobcsb", bufs=1)
                        nc.vector.tensor_copy(bc_sb[:], bc_ps[:])
                        nc.vector.tensor_tensor(
                            aoT8[po:po + 64, p * TL:(p + 1) * TL],
                            ao_ps[hh][0:64, :], bc_sb[:], ALU.mult)

                # ---------- proj + residual ----------
                xres = []
                for m in range(DT):
                    slab = wpool.tile([128, DT * 128], F8, tag="qkslab",
                                      name="projslab")
                    nc.sync.dma_start(
                        slab[:], wproj_d[:, m * 1024:(m + 1) * 1024])
                    ps = ps_bc.tile([128, TL], F32, tag="bc", name="proj")
                    for j in range(DT // 2):
                        _dr(nc, ps[:], _wpair(slab, j), _apair(aoT8, j),
                            j == 0, j == DT // 2 - 1)
                    xr = ppool.tile([128, TL], F32, tag=f"xres{m}",
                                    name=f"xres{m}")
                    nc.vector.scalar_tensor_tensor(
                        xr[:], ps[:], bproj_sb[:, m:m + 1], xT_sb[m][:],
                        ALU.add, ALU.add)
                    xres.append(xr)

            # ---------- LN2 (fp8 + bf16 outputs) ----------
            layernorm_cm(xres, ln2g_sb, ln2b_sb, x2f8, x2b)

            # ---------- router (bf16 path) ----------
            logit_ps = ps_row.tile([E, TL], F32, tag="row", name="logit")
            for j in range(DT):
                w = spool.tile([128, E], BF16, tag="wroute", name="wroute")
                nc.sync.dma_start(w[:], wroute_d[j * 128:(j + 1) * 128, :])
                _mm(nc, logit_ps[:], w[:], x2b[:, j * TL:(j + 1) * TL],
                    j == 0, j == DT - 1)
            nlin_ps = ps_row.tile([E, TL], F32, tag="row", name="nlin")
            for j in range(DT):
                w = spool.tile([128, E], BF16, tag="wnoise", name="wnoise")
                nc.sync.dma_start(w[:], wnoise_d[j * 128:(j + 1) * 128, :])
                _mm(nc, nlin_ps[:], w[:], x2b[:, j * TL:(j + 1) * TL],
                    j == 0, j == DT - 1)
            logits = spool.tile([E, TL], F32, tag="logits", name="logits",
                                bufs=1)
            nc.vector.tensor_scalar(logits[:], logit_ps[:],
                                    broute_sb[:, 0:1], None, ALU.add)
            spe = spool.tile([E, TL], BF16, tag="softpe", name="softpe",
                             bufs=1)
            nc.scalar.activation(spe[:], nlin_ps[:], AF.Exp,
                                 bias=bnoise_sb[:, 0:1])
            spe1 = spool.tile([E, TL], BF16, tag="softpe1", name="softpe1",
                              bufs=1)
            nc.vector.tensor_scalar_add(spe1[:], spe[:], 1.0)
            sp = spool.tile([E, TL], BF16, tag="softp", name="softp",
                            bufs=1)
            nc.scalar.activation(sp[:], spe1[:], AF.Ln)
            noiseT_sb = spool.tile([E, TL], BF16, tag="noiseTs",
                                   name="noiseTs", bufs=1)
            nc.sync.dma_start(noiseT_sb[:], noiseT_d[:])
            nsp = spool.tile([E, TL], BF16, tag="nsp", name="nsp", bufs=1)
            nc.vector.tensor_tensor(nsp[:], noiseT_sb[:], sp[:], ALU.mult)
            noisy_cm = spool.tile([E, TL], F32, tag="noisycm", name="noisycm",
                                  bufs=1)
            nc.vector.tensor_tensor(noisy_cm[:], nsp[:], logits[:], ALU.add)

            # ---------- top-2 gates (TM) ----------
            noisy8 = ppool.tile([128, 8 * NT], F32, tag="noisy8",
                                name="noisy8")
            nc.vector.memset(noisy8[:], -1e30)
            m8 = ppool.tile([128, 8 * NT], F32, tag="m8", name="m8")
            gate = ppool.tile([128, E * NT], F32, tag="gate", name="gate")
            mask = ppool.tile([128, E * NT], F32, tag="mask", name="mask")
            geT = ppool.tile([E, TL], F32, tag="geT", name="geT")
            geb = ppool.tile([E, TL], BF16, tag="geb", name="geb")
            cnt_sb = ppool.tile([1, NT * E], F32, tag="cntsb", name="cntsb")
            for j in range(NT):
                tr_ps = ps_bc.tile([128, E], F32, tag="bc", name="ntr")
                nc.tensor.matmul(tr_ps[:],
                                 noisy_cm[:, j * 128:(j + 1) * 128],
                                 eye_sb[0:E, 0:E], is_transpose=True,
                                 start=True, stop=True)
                nc.vector.tensor_copy(noisy8[:, 8 * j:8 * j + E], tr_ps[:])
            for j in range(NT):
                nm = noisy8[:, 8 * j:8 * j + E]
                nc.vector.max(m8[:, 8 * j:8 * j + 8],
                              noisy8[:, 8 * j:8 * j + 8])
                v1 = m8[:, 8 * j:8 * j + 1]
                v2 = m8[:, 8 * j + 1:8 * j + 2]
                oh1 = spool.tile([128, E], F32, tag="oh1", name="oh1")
                nc.vector.tensor_scalar(oh1[:], nm, v1, None, ALU.is_ge)
                msk = mask[:, E * j:E * (j + 1)]
                nc.vector.tensor_scalar(msk, nm, v2, None, ALU.is_ge)
                oh2 = spool.tile([128, E], F32, tag="oh2", name="oh2")
                nc.vector.tensor_tensor(oh2[:], msk, oh1[:], ALU.subtract)
                negv1 = spool.tile([128, 1], F32, tag="negv1", name="negv1")
                nc.vector.tensor_scalar_mul(negv1[:], v1, -1.0)
                p2 = spool.tile([128, 1], F32, tag="p2", name="p2")
                nc.scalar.activation(p2[:], v2, AF.Exp, bias=negv1[:])
                dden = spool.tile([128, 1], F32, tag="dden", name="dden")
                nc.vector.tensor_scalar_add(dden[:], p2[:], 1.0)
                rd = spool.tile([128, 1], F32, tag="rd", name="rd")
                nc.vector.reciprocal(rd[:], dden[:])
                gnum = spool.tile([128, E], F32, tag="gnum", name="gnum")
                nc.vector.tensor_scalar(gnum[:], oh2[:], p2[:], None,
                                        ALU.mult)
                gnum2 = spool.tile([128, E], F32, tag="gnum2", name="gnum2")
                nc.vector.tensor_tensor(gnum2[:], gnum[:], oh1[:], ALU.add)
                nc.vector.tensor_scalar(gate[:, E * j:E * (j + 1)],
                                        gnum2[:], rd[:], None, ALU.mult)
                cps = ps_row.tile([1, E], F32, tag="row", name="cnt")
                _mm(nc, cps[:], ones_sb[:, 0:1], msk, True, True, F32)
                nc.vector.tensor_copy(cnt_sb[0:1, E * j:E * (j + 1)], cps[:])

            # total counts -> all-gather
            tot = spool.tile([1, E], F32, tag="cnttot", name="cnttot",
                             bufs=1)
            nc.vector.tensor_tensor(tot[:], cnt_sb[0:1, 0:E],
                                    cnt_sb[0:1, E:2 * E], ALU.add)
            nc.vector.tensor_tensor(tot[:], tot[:], cnt_sb[0:1, 2 * E:3 * E],
                                    ALU.add)
            nc.vector.tensor_tensor(tot[:], tot[:], cnt_sb[0:1, 3 * E:4 * E],
                                    ALU.add)
            nc.sync.dma_start(cnt_in[:], tot[:])
            nc.gpsimd.collective_compute(
                "AllGather", ALU.bypass, replica_groups=rg_all,
                ins=[cnt_in[:].opt()], outs=[cnt_out[:].opt()])

            # ---------- MoE hidden + MLP hidden (overlaps counts AG) ------
            for e in range(E):
                for hmi in range(MOEH // 128):
                    me = 2 * e + hmi
                    slab = wpool.tile([128, DT * 128], F8, tag="qkslab",
                                      name="e1slab")
                    nc.sync.dma_start(
                        slab[:], we1_d[:, me * 1024:(me + 1) * 1024])
                    ps = ps_bc.tile([128, TL], F32, tag="bc", name="hmoe")
                    for j in range(DT // 2):
                        _dr(nc, ps[:], _wpair(slab, j), _apair(x2f8, j),
                            j == 0, j == DT // 2 - 1)
                    nc.scalar.activation(
                        Hmoe8[:, me * TL:(me + 1) * TL], ps[:], AF.Gelu,
                        bias=be1_sb[:, me:me + 1])
            for m in range(MLPH // 128):
                slab = wpool.tile([128, DT * 128], BF16, tag="m1slab",
                                  name="m1slab")
                nc.sync.dma_start(
                    slab[:], wmlp1_d[:, m * 1024:(m + 1) * 1024])
                ps = ps_bc.tile([128, TL], F32, tag="bc", name="hm")
                for kk in range(DT):
                    _mm(nc, ps[:], slab[:, kk * 128:(kk + 1) * 128],
                        x2b[:, kk * TL:(kk + 1) * TL],
                        kk == 0, kk == DT - 1)
                nc.scalar.activation(Hmb[:, m * TL:(m + 1) * TL], ps[:],
                                     AF.Gelu, bias=bmlp1_sb[:, m:m + 1])

            # ---------- ranks / keep / gate_eff ----------
            cntg = spool.tile([NC, E], F32, tag="cntg", name="cntg", bufs=1)
            nc.sync.dma_start(cntg[:], cnt_out[:])
            off_ps = ps_row.tile([1, E], F32, tag="row", name="off")
            _mm(nc, off_ps[:], wpfx_sb[:], cntg[:], True, True, F32)
            car = spool.tile([1, E * NT], F32, tag="car", name="car", bufs=1)
            nc.vector.tensor_copy(car[:, 0:E], off_ps[:])
            for j in range(1, NT):
                nc.vector.tensor_tensor(car[:, E * j:E * (j + 1)],
                                        car[:, E * (j - 1):E * j],
                                        cnt_sb[0:1, E * (j - 1):E * j],
                                        ALU.add)
            ge_tm = ppool.tile([128, E * NT], F32, tag="getm", name="getm")
            for j in range(NT):
                rk_ps = ps_bc.tile([128, E], F32, tag="bc", name="rank")
                _mm(nc, rk_ps[:], utri_sb[:],
                    mask[:, E * j:E * (j + 1)], True, False, F32)
                _mm(nc, rk_ps[:], ones_sb[0:1, :],
                    car[:, E * j:E * (j + 1)], False, True, F32)
                keep = spool.tile([128, E], F32, tag="keep", name="keep")
                nc.vector.tensor_scalar(keep[:], rk_ps[:], float(CAP), None,
                                        ALU.is_lt)
                nc.vector.tensor_tensor(ge_tm[:, E * j:E * (j + 1)],
                                        gate[:, E * j:E * (j + 1)],
                                        keep[:], ALU.mult)
            for j in range(NT):
                tr_ps = ps_bc.tile([E, 128], F32, tag="bc", name="getr")
                nc.tensor.matmul(tr_ps[:], ge_tm[:, E * j:E * (j + 1)],
                                 eye_sb[:, :], is_transpose=True,
                                 start=True, stop=True)
                nc.vector.tensor_copy(geT[:, j * 128:(j + 1) * 128], tr_ps[:])
            nc.vector.tensor_copy(geb[:], geT[:])

            # gate the MoE hidden
            for e in range(E):
                bc_ps = ps_bc.tile([128, TL], F32, tag="bc", name="gbc")
                _mm(nc, bc_ps[:], gsel_sb[:, e * 128:(e + 1) * 128],
                    geb[:], True, True)
                bc_sb = spool.tile([128, TL], BF16, tag="gbcsb", name="gbcsb",
                                   bufs=2)
                nc.vector.tensor_copy(bc_sb[:], bc_ps[:])
                for hmi in range(MOEH // 128):
                    me = 2 * e + hmi
                    nc.vector.tensor_tensor(Hg8[:, me * TL:(me + 1) * TL],
                                            Hmoe8[:, me * TL:(me + 1) * TL],
                                            bc_sb[:], ALU.mult)

            # ---------- output GEMM: moe + be2 + mlp, fused accum ----------
            for m in range(DT):
                slab = wpool.tile([128, 8 * 128], F8, tag="outslab",
                                  name="outslab")
                nc.sync.dma_start(
                    slab[:], wout_d[:, m * 1024:(m + 1) * 1024])
                slab2 = wpool.tile([128, 32 * 128], BF16, tag="out2slab",
                                   name="out2slab")
                nc.sync.dma_start(
                    slab2[:], wm2_d[:, m * 4096:(m + 1) * 4096])
                ps = ps_bc.tile([128, TL], F32, tag="bc", name="out")
                for j in range(4):           # we2 tile pairs (e, hmi)
                    _dr(nc, ps[:], _wpair(slab, j), _apair(Hg8, j),
                        j == 0, False)
                _mm(nc, ps[:], be2_sb[:, m * 128:(m + 1) * 128],
                    geb[:], False, False)
                for kk in range(32):         # mlp2 tiles (bf16)
                    _mm(nc, ps[:], slab2[:, kk * 128:(kk + 1) * 128],
                        Hmb[:, kk * TL:(kk + 1) * TL],
                        False, kk == 31)
                o = spool.tile([128, TL], F32, tag="outsb", name="outsb",
                               bufs=2)
                nc.vector.scalar_tensor_tensor(
                    o[:], ps[:], bmlp2_sb[:, m:m + 1], xres[m][:],
                    ALU.add, ALU.add)
                nc.sync.dma_start(out_d[m * 128:(m + 1) * 128, :], o[:])

    nc.compile()
    return nc


def _tile_lhst(w, n_k, n_m):
    # w: [n_k*128, n_m*128] -> [128, n_m, n_k, 128] -> [128, n_m*n_k*128]
    kdim, mdim = w.shape
    return np.ascontiguousarray(
        w.reshape(n_k, 128, n_m, 128).transpose(1, 2, 0, 3)
        .reshape(128, n_m * n_k * 128))


def _prep_inputs(inputs):
    f32 = lambda a: np.ascontiguousarray(np.asarray(a, np.float32))
    bf = lambda a: np.ascontiguousarray(
        np.asarray(a, np.float32).astype(ml_dtypes.bfloat16))
    f8 = lambda a: np.ascontiguousarray(
        np.clip(np.asarray(a, np.float32), -240, 240)
        .astype(ml_dtypes.float8_e4m3))
    x = f32(inputs["x"]).reshape(T, D)
    noise = f32(inputs["noise"]).reshape(T, E)
    w_qkv = np.asarray(inputs["w_qkv"], np.float32)
    wqkT = w_qkv[:2 * D].T                       # [D, 2048]
    wvT = w_qkv[2 * D:].T                        # [D, D]
    wprojT = np.asarray(inputs["w_proj"], np.float32).T
    we1 = np.asarray(inputs["we1"], np.float32)  # [E, D, MOEH]
    we2 = np.asarray(inputs["we2"], np.float32)  # [E, MOEH, D]
    wmlp1 = np.asarray(inputs["w_mlp1"], np.float32)   # [D, MLPH]
    wmlp2 = np.asarray(inputs["w_mlp2"], np.float32)   # [MLPH, D]

    # we1 slabs: m-index = e*2+hmi over [D, 256] each
    we1_flat = np.concatenate([we1[e] for e in range(E)], 1)  # [D, E*MOEH]
    # wout: per m, 8 we2 tiles (e,hmi); wm2: per m, 32 wmlp2 tiles
    we2_l = we2.reshape(E, 2, 128, DT, 128).transpose(2, 3, 0, 1, 4) \
        .reshape(128, DT * 8 * 128)
    wm2_l = wmlp2.reshape(32, 128, DT, 128).transpose(1, 2, 0, 3) \
        .reshape(128, DT * 32 * 128)

    shared = dict(
        wqk_l=f8(_tile_lhst(wqkT, DT, 16)),
        wv_l=f8(np.ascontiguousarray(
            wvT.reshape(DT, 128, D).transpose(1, 0, 2).reshape(128, DT * D))),
        wproj_l=f8(_tile_lhst(wprojT, DT, DT)),
        wmlp1_l=bf(_tile_lhst(wmlp1, DT, 32)),
        we1_l=f8(_tile_lhst(we1_flat, DT, 8)),
        wout_l=f8(we2_l),
        wm2_l=bf(wm2_l),
        wroute=bf(inputs["w_route"]),
        wnoise=bf(inputs["w_noise"]),
        be2=bf(inputs["be2"]),
        ln1g=f32(inputs["ln1_g"]).reshape(D, 1),
        ln1b=f32(inputs["ln1_b"]).reshape(D, 1),
        ln2g=f32(inputs["ln2_g"]).reshape(D, 1),
        ln2b=f32(inputs["ln2_b"]).reshape(D, 1),
        bproj=f32(inputs["b_proj"]).reshape(D, 1),
        broute=f32(inputs["b_route"]).reshape(E, 1),
        bnoise=f32(inputs["b_noise"]).reshape(E, 1),
        be1=f32(inputs["be1"]).reshape(E * MOEH, 1),
        bmlp1=f32(inputs["b_mlp1"]).reshape(MLPH, 1),
        bmlp2=f32(inputs["b_mlp2"]).reshape(D, 1),
        ones128=np.ones((128, 128), np.float32),
        eye128=np.eye(128, dtype=np.float32),
        utri128=np.triu(np.ones((128, 128), np.float32), 1),
        gsel=np.repeat(np.eye(E, dtype=np.float32), 128, 1)
            .astype(ml_dtypes.bfloat16),
        ones128b=np.ones((128, 128), ml_dtypes.bfloat16),
    )
    in_maps = []
    for c in range(NC):
        m = dict(shared)
        m["xT"] = bf(x[c * TL:(c + 1) * TL].T)
        m["noiseT"] = bf(noise[c * TL:(c + 1) * TL].T)
        m["wpfx"] = (np.arange(NC) < c).astype(np.float32).reshape(NC, 1)
        in_maps.append(m)
    return in_maps


def _run(inputs, trace=False):
    if "nc" not in _cache:
        _cache["nc"] = _build()
    nc = _cache["nc"]
    in_maps = _prep_inputs(inputs)
    res = run_bass_kernel_spmd(nc, in_maps, core_ids=list(range(NC)),
                               trace=trace)
    _cache["last_res"] = res
    shards = [res.results[c]["out"] for c in range(NC)]   # each [D, TL]
    out = np.concatenate([np.asarray(s, np.float32).T for s in shards],
                         0).reshape(B, N, D)
    return out.astype(np.float32), res.exec_time_ns


def kernel(**inputs):
    out, _ = _run(inputs, trace=False)
    return out
